# revision 47
# baseline (speedup 1.0000x reference)
"""Single-head causal attention (B=256, T=256, C=1024, D=64) on 8 TRN2 NeuronCores.

Data-parallel over batch (32 batches/core). The per-core x traffic is the
roofline (~16.8MB at ~360B/ns of serialized DMA-engine time = ~47us); the
schedule keeps the DMA stream gapless and holds PE/DVE/ACT/Pool well under
the DMA cadence:

  * x ships as an fp8-e4m3 hi/lo pair (x = xh + xl, xl the unscaled
    residual), pre-transposed to partition-major [C, T]. Same bytes as bf16,
    but the projections run in DoubleRow fp8 mode (4x bf16 PE throughput),
    contracting two 128-chunks per instruction:
        q|k^T [128,T]: xh@wh + xl@wh            (2 terms, 8 matmuls/batch)
        v     [T,D]:   xh@wh + xl@wh + xh@wl    (3 terms, 24 matmuls/batch)
    (v keeps the third term -- its error feeds the output directly; q/k only
    perturb softmax weights, rel err ~9e-3 total vs the 2e-2 gate.)
    Weights are host-prescaled by 8 so their fp8 hi/lo split stays in the
    normal range; the 8x comes out in the exp scale and the fused
    denominator column (memset 8.0).
  * Superbatch pipeline (2 batches per stage), per iteration sb:
    qk(sb) -> [q copy on DVE | k copy on ACT] -> v(sb) ->
    scores^T(sb-1) in bf16 -> one ACT exp per batch -> Pool affine_select
    causal masks (two diagonal quadrants only) -> finale(sb-2):
    o' [T,2,D+1] = e^T.T @ [v | 8] + DVE reciprocal/multiply into a bf16
    staging tile (host upcasts to f32). Splitting the PSUM->SBUF q/k copies
    across DVE and ACT keeps the copy chain off the critical loop.
  * DMA: one 1MB x load per superbatch (8KB/partition contiguous), one 64KB
    store per superbatch, split wqk/wv weight loads; ~36 DMA instructions,
    all with >=512B descriptors (no sub-512B latency penalty). First/last
    loads are split per batch to shorten pipeline fill/drain.
"""

import numpy as np
import ml_dtypes

import concourse.bacc as bacc
import concourse.mybir as mybir
import concourse.tile as tile
from concourse.bass_utils import run_bass_kernel_spmd

B, T, C, D = 256, 256, 1024, 64
NCORES = 8
BPC = B // NCORES  # batches per core
NSB = BPC // 2  # superbatches (2 batches per DMA)
CCH = C // 128  # contraction chunks
NCP = CCH // 2  # chunk pairs (DoubleRow contracts 2 chunks/instruction)
WS = 8.0  # host weight prescale; keeps fp8 weight splits in normal range
SCALE = float(C) ** -0.5

BF16 = mybir.dt.bfloat16
F32 = mybir.dt.float32
F8 = mybir.dt.float8e4
E4M3 = ml_dtypes.float8_e4m3
DR = mybir.MatmulPerfMode.DoubleRow

TRACE = False
LAST_RESULT = None

# (x half, w half) term order: hi@hi, lo@hi, hi@lo
TERMS = ((0, 0), (1, 0), (0, 1))


def _build(pf=2, qk_terms=2, v_terms=3, mask_eng='affine', vcopy_eng='act'):
    nc = bacc.Bacc(
        "TRN2", target_bir_lowering=False, debug=False, num_devices=NCORES
    )
    # [sb, partition, batch-in-sb, hi/lo, chunk, t]
    xhl = nc.dram_tensor("xhl", [NSB, 128, 2, 2, CCH, T], F8, kind="ExternalInput")
    # [partition, chunk, hi/lo, .] : wqk = [Wq|Wk]*8, wv = Wv*8
    wqk_d = nc.dram_tensor("wqk_d", [128, CCH, 2, 128], F8, kind="ExternalInput")
    wv_d = nc.dram_tensor("wv_d", [128, CCH, 2, 64], F8, kind="ExternalInput")
    # [sb, partition, batch-in-sb, t-tile, d]
    out = nc.dram_tensor("out", [NSB, 128, 2, 2, D], BF16, kind="ExternalOutput")

    with tile.TileContext(nc) as tc:
        with (
            tc.tile_pool(name="singles", bufs=1) as singles,
            tc.tile_pool(name="xp", bufs=pf + 1) as xp,
            tc.tile_pool(name="sbp", bufs=3) as sbp,
            tc.tile_pool(name="ep", bufs=8) as ep,
            tc.tile_pool(name="vp", bufs=6) as vp,
            tc.tile_pool(name="stp", bufs=5) as stp,
            tc.tile_pool(name="rp", bufs=2) as rp,
            tc.tile_pool(name="qk_ps", bufs=2, space="PSUM") as qk_psp,
            tc.tile_pool(name="sc_ps", bufs=3, space="PSUM") as sc_psp,
            tc.tile_pool(name="v_ps", bufs=2, space="PSUM") as v_psp,
            tc.tile_pool(name="o_ps", bufs=1, space="PSUM") as o_psp,
        ):
            # qk weights load first: they gate the very first projection
            wqk_sb = singles.tile([128, CCH, 2, 128], F8)
            nc.sync.dma_start(wqk_sb, wqk_d[:])
            wv_sb = singles.tile([128, CCH, 2, D], F8)

            # causal triangle (1 where s <= t within a 128-tile) built once;
            # masking is then a tensor-tensor multiply on any engine
            tri = singles.tile([128, 128], BF16)
            nc.gpsimd.memset(tri, 1.0)
            nc.gpsimd.affine_select(
                out=tri, in_=tri,
                compare_op=mybir.AluOpType.is_ge,
                fill=0.0, base=0, pattern=[[1, 128]], channel_multiplier=-1,
            )

            xt_tiles = {}

            def load_sb(k, split=False):
                t = xp.tile([128, 2, 2, CCH, T], F8, tag="xt")
                if split:
                    # per-batch halves: first batch's data (and compute)
                    # lands ~1.5us earlier at the pipeline head/tail
                    nc.sync.dma_start(t[:, 0], xhl[k][:, 0])
                    nc.sync.dma_start(t[:, 1], xhl[k][:, 1])
                else:
                    nc.sync.dma_start(t, xhl[k])
                xt_tiles[k] = t

            stages = {}

            def final_stage(sb, expT0, expT1, v_sb):
                """o' matmuls + softmax normalization for superbatch sb
                (emitted two superbatches late)."""
                o2 = o_psp.tile([128, 2, 2, D + 1], F32, tag="o_ps")
                for bi, expT in ((0, expT0), (1, expT1)):
                    nc.tensor.matmul(
                        o2[:, bi, 0], lhsT=expT[:, 0:128], rhs=v_sb[:, bi, 0],
                        start=True, stop=True,
                    )
                    nc.tensor.matmul(
                        o2[:, bi, 1], lhsT=expT[:, 128:256], rhs=v_sb[:, bi, 0],
                        start=True, stop=False,
                    )
                    nc.tensor.matmul(
                        o2[:, bi, 1], lhsT=expT[:, 256:384], rhs=v_sb[:, bi, 1],
                        start=False, stop=True,
                    )
                stages[sb] = stp.tile(
                    [128, 2, 2, D], BF16, tag="stage", name="stage"
                )
                # recip to SBUF first: engines may read only ONE PSUM
                # operand per instruction, and Pool can't read PSUM at all
                recip = rp.tile([128, 2, 2], F32, tag="recip")
                nc.vector.reciprocal(recip, o2[:, :, :, D])
                nc.vector.tensor_tensor(
                    stages[sb],
                    o2[:, :, :, 0:D],
                    recip[:, :, :, None].to_broadcast((128, 2, 2, D)),
                    mybir.AluOpType.mult,
                )

            def scores_stage(sb, q_sb, k_sb, v_sb):
                """scores^T + exp + causal mask for both batches of sb
                (emitted one superbatch late)."""
                expTs = []
                for bi in range(2):
                    # scores^T packed [128, 384]: cols 0:256 = (s<128, all t),
                    # 256:384 = (s>=128, t>=128); (s>=128, t<128) fully masked
                    sc_ps = sc_psp.tile([128, 3 * 128], F32, tag="sc")
                    nc.tensor.matmul(
                        sc_ps[:, 0:T],
                        lhsT=k_sb[:, bi, 0:128],
                        rhs=q_sb[:, bi],
                        start=True, stop=True,
                    )
                    nc.tensor.matmul(
                        sc_ps[:, T : T + 128],
                        lhsT=k_sb[:, bi, 128:T],
                        rhs=q_sb[:, bi, 128:T],
                        start=True, stop=True,
                    )
                    expT = ep.tile([128, 3 * 128], BF16, tag="expT")
                    nc.scalar.activation(
                        expT, sc_ps,
                        func=mybir.ActivationFunctionType.Exp,
                        scale=SCALE / (WS * WS),
                    )
                    for qi, quad in enumerate((0, 256)):
                        if mask_eng == 'affine':
                            nc.gpsimd.affine_select(
                                out=expT[:, quad : quad + 128],
                                in_=expT[:, quad : quad + 128],
                                compare_op=mybir.AluOpType.is_ge,
                                fill=0.0, base=0, pattern=[[1, 128]],
                                channel_multiplier=-1,
                            )
                            continue
                        if mask_eng == 'dve':
                            eng = nc.vector
                        elif mask_eng == 'pool':
                            eng = nc.gpsimd
                        else:  # mix: one quadrant each
                            eng = nc.vector if qi == 0 else nc.gpsimd
                        eng.tensor_tensor(
                            expT[:, quad : quad + 128],
                            expT[:, quad : quad + 128],
                            tri,
                            mybir.AluOpType.mult,
                        )
                    expTs.append(expT)
                return (sb, expTs[0], expTs[1], v_sb)

            load_sb(0, split=True)
            nc.sync.dma_start(wv_sb, wv_d[:])
            load_sb(1, split=True)
            for k in range(2, min(pf, NSB)):
                load_sb(k)

            pend_sc = None  # superbatch sb-1: awaiting scores/exp/mask
            fin_q = []  # superbatches sb-2, sb-3: awaiting o'/normalize
            for sb in range(NSB):
                if sb + pf < NSB:
                    load_sb(sb + pf, split=(sb + pf == NSB - 1))
                if sb >= 4:
                    nc.sync.dma_start(out[sb - 4], stages.pop(sb - 4))
                xt = xt_tiles[sb]

                # q|k projections for both batches: one 2KB PSUM bank,
                # two accumulation groups of DoubleRow matmuls. scores(sb-1)
                # is emitted BETWEEN the groups so its exp/mask chain starts
                # ~1.5us earlier in the iteration (it is the longest serial
                # chain feeding next iteration's finale).
                qk_ps = qk_psp.tile([128, 2, T], F32, tag="qk")
                n = qk_terms * NCP

                def qk_group(bi):
                    i = 0
                    for xh_, wh_ in TERMS[:qk_terms]:
                        for cp in range(NCP):
                            nc.tensor.matmul(
                                qk_ps[:, bi],
                                lhsT=wqk_sb[:, 2 * cp : 2 * cp + 2, wh_],
                                rhs=xt[:, bi, xh_, 2 * cp : 2 * cp + 2, :],
                                start=(i == 0),
                                stop=(i == n - 1),
                                perf_mode=DR,
                            )
                            i += 1

                qk_group(0)
                qk_group(1)

                # q/k copies first in the DVE/ACT programs: they are the
                # critical arm feeding this superbatch's scores
                q_sb = sbp.tile([64, 2, T], BF16, tag="q_sb")
                k_sb = sbp.tile([64, 2, T], BF16, tag="k_sb")
                nc.vector.tensor_copy(q_sb, qk_ps[0:64])
                nc.scalar.copy(k_sb, qk_ps[64:128])


                # v projections: all four groups in one PSUM bank, single
                # fused ACT copy (emitted before exp in the ACT program)
                v_sb = vp.tile([128, 2, 2, D + 1], BF16, tag="v")
                v_ps = v_psp.tile([128, 2, 2, D], F32, tag="v_ps")
                n = v_terms * NCP
                for bi in range(2):
                    for st in range(2):
                        i = 0
                        for xh_, wh_ in TERMS[:v_terms]:
                            for cp in range(NCP):
                                nc.tensor.matmul(
                                    v_ps[:, bi, st],
                                    lhsT=xt[
                                        :, bi, xh_, 2 * cp : 2 * cp + 2,
                                        st * 128 : (st + 1) * 128,
                                    ],
                                    rhs=wv_sb[:, 2 * cp : 2 * cp + 2, wh_],
                                    start=(i == 0),
                                    stop=(i == n - 1),
                                    perf_mode=DR,
                                )
                                i += 1
                if vcopy_eng == 'pool':
                    nc.gpsimd.tensor_copy(v_sb[:, :, :, 0:D], v_ps)
                elif vcopy_eng == 'dve':
                    nc.vector.tensor_copy(v_sb[:, :, :, 0:D], v_ps)
                else:
                    nc.scalar.copy(v_sb[:, :, :, 0:D], v_ps)
                nc.gpsimd.memset(v_sb[:, :, :, D : D + 1], WS)

                # scores(sb-1): operands copied last iteration -> ready now
                if pend_sc is not None:
                    fin_q.append(scores_stage(*pend_sc))
                # finale(sb-2): expT masked last iteration -> ready now
                if len(fin_q) >= 2:
                    final_stage(*fin_q.pop(0))

                pend_sc = (sb, q_sb, k_sb, v_sb)

            # drain: scores(15), finals(14..15), stores for sb 12..15
            fin_q.append(scores_stage(*pend_sc))
            nc.sync.dma_start(out[NSB - 4], stages.pop(NSB - 4))
            final_stage(*fin_q.pop(0))
            nc.sync.dma_start(out[NSB - 3], stages.pop(NSB - 3))
            final_stage(*fin_q.pop(0))
            nc.sync.dma_start(out[NSB - 2], stages.pop(NSB - 2))
            last = stages.pop(NSB - 1)
            nc.sync.dma_start(out[NSB - 1][:, 0], last[:, 0])
            nc.sync.dma_start(out[NSB - 1][:, 1], last[:, 1])
    nc.compile()
    return nc


def _pack_inputs(x, Wq, Wk, Wv):
    """Host-side layout/dtype prep: per-core [NSB,128,2,2,CCH,T] fp8 hi/lo x
    and the shared packed weight blob."""
    xt = np.ascontiguousarray(np.transpose(x, (0, 2, 1)))  # [B, C, T] f32
    xh = xt.astype(E4M3)
    xl = (xt - xh.astype(np.float32)).astype(E4M3)
    # [B, C, T] -> [B//2, 2, CCH, 128, T] -> stack hl -> [NSB*8, 128, 2, 2, CCH, T]
    def pack(a):
        return a.reshape(B // 2, 2, CCH, 128, T)
    ph, pl = pack(xh), pack(xl)
    xhl = np.stack([ph, pl], axis=2)  # [B//2, 2, 2, CCH, 128, T]
    xhl = np.ascontiguousarray(xhl.transpose(0, 4, 1, 2, 3, 5))

    def pack_w(W, m):
        w8 = W * WS
        wh = w8.astype(E4M3)
        wl = (w8 - wh.astype(np.float32)).astype(E4M3)
        return np.ascontiguousarray(
            np.stack(
                [wh.reshape(CCH, 128, m), wl.reshape(CCH, 128, m)], axis=2
            ).transpose(1, 0, 2, 3)
        )

    wqk = pack_w(np.concatenate([Wq, Wk], axis=1), 128)
    wv = pack_w(Wv, D)
    return np.ascontiguousarray(xhl), wqk, wv


def kernel(x: np.ndarray, Wq: np.ndarray, Wk: np.ndarray, Wv: np.ndarray) -> np.ndarray:
    global LAST_RESULT
    x = np.asarray(x, dtype=np.float32)
    Wq = np.asarray(Wq, dtype=np.float32)
    Wk = np.asarray(Wk, dtype=np.float32)
    Wv = np.asarray(Wv, dtype=np.float32)

    xhl, wqk, wv = _pack_inputs(x, Wq, Wk, Wv)

    nc = _build()
    in_maps = [
        {"xhl": xhl[i * NSB : (i + 1) * NSB], "wqk_d": wqk, "wv_d": wv}
        for i in range(NCORES)
    ]
    res = run_bass_kernel_spmd(
        nc, in_maps, core_ids=list(range(NCORES)), trace=TRACE
    )
    LAST_RESULT = res
    # [NSB, 128, 2, 2, D] -> [NSB, 2, 2, 128, D] -> [BPC, T, D]
    outs = [
        np.ascontiguousarray(r["out"].transpose(0, 2, 3, 1, 4))
        .reshape(BPC, T, D)
        .astype(np.float32)
        for r in res.results
    ]
    return np.concatenate(outs, axis=0)


if __name__ == "__main__":
    x = np.random.randn(B, T, C).astype(np.float32)
    Wq = np.random.randn(C, D).astype(np.float32) * (C**-0.5)
    Wk = np.random.randn(C, D).astype(np.float32) * (C**-0.5)
    Wv = np.random.randn(C, D).astype(np.float32) * (C**-0.5)
    o = kernel(x, Wq, Wk, Wv)
    print(o.shape, o.dtype)
